# revision 3
# baseline (speedup 1.0000x reference)
"""Trainium2 Bass kernel for the PraxisMemory scatter_memory problem.

Strategy (8 NeuronCores, SPMD single launch):
  - Vocab-sharded fp16 logits GEMM (brain_w.T shard [512,4000] per core) with
    on-device exp + accumulation -> per-core partial sum-exp per token.
    Target-logit dot product kept in exact f32 on the vector engine.
  - Token-similarity matrix (sims) row-sharded across cores via float32r matmuls.
  - Storage MLP applied to every query token row (row-sharded, 64 rows + one
    zero row per core) so the data-dependent retrieval later is a pure gather.
  - Host (numpy, f64) does the tiny decision logic: logsumexp combine, windowed
    threshold, boundary refinement via prefix-sum modularity/conductance deltas,
    event packing into memory slots, top-k retrieval, and final concat.
"""

import os
import numpy as np

import concourse.bacc as bacc
import concourse.mybir as mybir
from concourse.tile import TileContext
from concourse.bass_utils import run_bass_kernel_spmd

F32 = mybir.dt.float32
F32R = mybir.dt.float32r
F16 = mybir.dt.float16
AF = mybir.ActivationFunctionType
AX = mybir.AxisListType

B, S, D, V = 2, 256, 512, 32000
W_WIN, GAMMA = 20, 2.0
MEM_LEN, NUM_MEM, K_SIM, K_CONT = 16, 256, 8, 4
NCORES = 8
VSH = V // NCORES           # 4000 vocab per core
NV, VT = 8, 500             # vocab tiles per core x tile width
TOK = B * S                 # 512 tokens
RSH = TOK // NCORES         # 64 token rows per core (MLP/tgt shard)
SSH = S // NCORES           # 32 sims rows per batch per core
NR = RSH + 4                # MLP rows per core: 64 query rows + 4 zero rows (aligned)

_NC = None
LAST_RESULTS = None


def _build_nc():
    nc = bacc.Bacc("TRN2", target_bir_lowering=False, debug=False)

    wt16 = nc.dram_tensor("wt16", [NV, 4, 128, VT], F16, kind="ExternalInput")
    qt16 = nc.dram_tensor("qt16", [4, 128, TOK], F16, kind="ExternalInput")
    qtf = nc.dram_tensor("qtf", [4, 128, TOK], F32R, kind="ExternalInput")
    qts = nc.dram_tensor("qts", [4, 128, 2 * SSH], F32R, kind="ExternalInput")
    mxt = nc.dram_tensor("mxt", [4, 128, NR], F32R, kind="ExternalInput")
    w1t = nc.dram_tensor("w1t", [4, 128, D], F32R, kind="ExternalInput")
    w2t = nc.dram_tensor("w2t", [4, 128, D], F32R, kind="ExternalInput")
    b1r = nc.dram_tensor("b1r", [1, D], F32R, kind="ExternalInput")
    b2r = nc.dram_tensor("b2r", [1, D], F32R, kind="ExternalInput")
    onesr = nc.dram_tensor("onesr", [1, 128], F32R, kind="ExternalInput")
    qrows = nc.dram_tensor("qrows", [RSH, D], F32, kind="ExternalInput")
    grows = nc.dram_tensor("grows", [RSH, D], F32, kind="ExternalInput")

    se_o = nc.dram_tensor("se_o", [128, 4], F32, kind="ExternalOutput")
    tl_o = nc.dram_tensor("tl_o", [RSH, 1], F32, kind="ExternalOutput")
    sims_o = nc.dram_tensor("sims_o", [B, SSH, S], F32, kind="ExternalOutput")
    y_o = nc.dram_tensor("y_o", [NR, D], F32, kind="ExternalOutput")

    with TileContext(nc) as tc:
        with tc.tile_pool(name="const", bufs=1) as cpool, \
             tc.tile_pool(name="wts", bufs=2) as wpool, \
             tc.tile_pool(name="work", bufs=3) as work, \
             tc.tile_pool(name="pslg", bufs=4, space="PSUM") as pslg, \
             tc.tile_pool(name="pssm", bufs=1, space="PSUM") as pssm:

            # ---- persistent loads ----
            qt16_sb = []
            qtf_sb = []
            qts_sb = []
            mxt_sb = []
            w1t_sb = []
            w2t_sb = []
            for k in range(4):
                t = cpool.tile([128, TOK], F16, tag=f"qt16_{k}")
                nc.sync.dma_start(t[:], qt16[k])
                qt16_sb.append(t)
                t = cpool.tile([128, TOK], F32R, tag=f"qtf_{k}")
                nc.sync.dma_start(t[:], qtf[k])
                qtf_sb.append(t)
                t = cpool.tile([128, 2 * SSH], F32R, tag=f"qts_{k}")
                nc.sync.dma_start(t[:], qts[k])
                qts_sb.append(t)
                t = cpool.tile([128, NR], F32R, tag=f"mxt_{k}")
                nc.sync.dma_start(t[:], mxt[k])
                mxt_sb.append(t)
                t = cpool.tile([128, D], F32R, tag=f"w1t_{k}")
                nc.sync.dma_start(t[:], w1t[k])
                w1t_sb.append(t)
                t = cpool.tile([128, D], F32R, tag=f"w2t_{k}")
                nc.sync.dma_start(t[:], w2t[k])
                w2t_sb.append(t)
            b1_sb = cpool.tile([1, D], F32R, tag="b1")
            nc.sync.dma_start(b1_sb[:], b1r[:])
            b2_sb = cpool.tile([1, D], F32R, tag="b2")
            nc.sync.dma_start(b2_sb[:], b2r[:])
            ones_sb = cpool.tile([1, 128], F32R, tag="ones")
            nc.sync.dma_start(ones_sb[:], onesr[:])
            qrows_sb = cpool.tile([RSH, D], F32, tag="qrows")
            nc.sync.dma_start(qrows_sb[:], qrows[:])
            grows_sb = cpool.tile([RSH, D], F32, tag="grows")
            nc.sync.dma_start(grows_sb[:], grows[:])

            # ---- target-logit dot (exact f32 on DVE) ----
            prod = work.tile([RSH, D], F32, tag="prod")
            nc.vector.tensor_mul(prod[:], qrows_sb[:], grows_sb[:])
            tl_sb = work.tile([RSH, 1], F32, tag="tl")
            nc.vector.reduce_sum(tl_sb[:], prod[:], axis=AX.X)
            nc.sync.dma_start(tl_o[:], tl_sb[:])

            # ---- logits GEMM (fp16) + exp-accumulate ----
            separts = [cpool.tile([128, NV], F32, tag=f"sep_{m}", name=f"sep_{m}") for m in range(4)]
            for n in range(NV):
                wtiles = []
                for k in range(4):
                    wt = wpool.tile([128, VT], F16, tag=f"wt_{k}")
                    nc.sync.dma_start(wt[:], wt16[n, k])
                    wtiles.append(wt)
                for m in range(4):
                    ps = pslg.tile([128, VT], F32, tag="lg")
                    for k in range(4):
                        nc.tensor.matmul(
                            ps[:], lhsT=qt16_sb[k][:, m * 128:(m + 1) * 128],
                            rhs=wtiles[k][:], start=(k == 0), stop=(k == 3))
                    et = work.tile([128, VT], F32, tag="exp")
                    nc.scalar.activation(et[:], ps[:], AF.Exp,
                                         accum_out=separts[m][:, n:n + 1])
            se_sb = cpool.tile([128, 4], F32, tag="se")
            for m in range(4):
                nc.vector.reduce_sum(se_sb[:, m:m + 1], separts[m][:], axis=AX.X)
            nc.sync.dma_start(se_o[:], se_sb[:])

            # ---- sims row-shard (f32r) ----
            for b in range(B):
                pss = pssm.tile([SSH, S], F32, tag="sims")
                for k in range(4):
                    nc.tensor.matmul(
                        pss[:], lhsT=qts_sb[k][:, b * SSH:(b + 1) * SSH],
                        rhs=qtf_sb[k][:, b * S:(b + 1) * S],
                        start=(k == 0), stop=(k == 3))
                so = work.tile([SSH, S], F32, tag="so")
                nc.vector.tensor_copy(so[:], pss[:])
                nc.sync.dma_start(sims_o[b], so[:])

            # ---- storage MLP on 64 query rows + zero rows (padded) ----
            ht_sb = []
            for e in range(4):
                psh = pssm.tile([128, NR], F32, tag="psh")
                for k in range(4):
                    nc.tensor.matmul(
                        psh[:], lhsT=w1t_sb[k][:, e * 128:(e + 1) * 128],
                        rhs=mxt_sb[k][:], start=(k == 0), stop=False)
                nc.tensor.matmul(psh[:], lhsT=b1_sb[:, e * 128:(e + 1) * 128],
                                 rhs=ones_sb[:, :NR], start=False, stop=True)
                hf = work.tile([128, NR], F32, tag="hf")
                nc.scalar.activation(hf[:], psh[:], AF.Relu)
                ht = cpool.tile([128, NR], F32R, tag=f"ht_{e}")
                nc.vector.tensor_copy(ht[:], hf[:])
                ht_sb.append(ht)
            psy = pssm.tile([NR, D], F32, tag="psy")
            for e in range(4):
                nc.tensor.matmul(psy[:], lhsT=ht_sb[e][:], rhs=w2t_sb[e][:],
                                 start=(e == 0), stop=False)
            nc.tensor.matmul(psy[:], lhsT=ones_sb[:, :NR], rhs=b2_sb[:],
                             start=False, stop=True)
            y_sb = work.tile([NR, D], F32, tag="y")
            nc.vector.tensor_copy(y_sb[:], psy[:])
            nc.sync.dma_start(y_o[:], y_sb[:])

    nc.compile()
    return nc


def _get_nc():
    global _NC
    if _NC is None:
        _NC = _build_nc()
    return _NC


# ---------------------------------------------------------------- host logic

def _windowed_threshold(sur):
    wv = np.lib.stride_tricks.sliding_window_view(sur, W_WIN, axis=1)
    t = wv.mean(-1) + GAMMA * wv.std(-1, ddof=1)
    return np.concatenate([np.repeat(t[:, :1], W_WIN - 1, axis=1), t], axis=1)


def _refine(sims, bnd):
    """Candidate boundary-removal decisions via analytic merge deltas (f64)."""
    refined = bnd.copy()
    for b in range(B):
        sim = sims[b].astype(np.float64)
        bd = bnd[b]
        deg = sim.sum(1)
        tot = deg.sum()
        com = np.cumsum(bd).astype(np.int64)
        nlast = int(com[-1])
        mask = (com[:, None] == com[None, :])
        expd = deg[:, None] * deg[None, :] / (tot + 1e-10)
        curQ = ((sim - expd) * mask).sum() / (tot + 1e-10)
        nmax = nlast + 1
        vol = np.zeros(nmax)
        intra = np.zeros(nmax)
        for c in np.unique(com):
            idx = com == c
            vol[c] = deg[idx].sum()
            intra[c] = sim[np.ix_(idx, idx)].sum()
        cuts = vol - intra
        minv = np.minimum(vol, tot - vol)
        cond = np.where(vol > 0, cuts / (minv + 1e-10), 0.0)
        cond_sum = cond.sum()
        curC = cond_sum / (nlast + 1.0)
        for t in np.flatnonzero(bd > 0):
            c = int(com[t])
            A = com == (c - 1)
            Bm = com == c
            S_AB = sim[np.ix_(A, Bm)].sum()
            degA = deg[A].sum()
            degB = deg[Bm].sum()
            newQ = curQ + 2.0 * (S_AB - degA * degB / (tot + 1e-10)) / (tot + 1e-10)
            volM = vol[c - 1] + vol[c]
            intraM = intra[c - 1] + intra[c] + 2.0 * S_AB
            condM = (volM - intraM) / (np.minimum(volM, tot - volM) + 1e-10)
            newC = (cond_sum - cond[c - 1] - cond[c] + condM) / float(nlast)
            if (newQ > curQ) or (newC < curC):
                refined[b, t] = 0.0
    return refined


def _topk_idx(values, k):
    """jax.lax.top_k semantics: descending values, ties -> lowest index."""
    n = len(values)
    order = np.lexsort((np.arange(n), -values))
    return order[:k]


def kernel(**inputs):
    global LAST_RESULTS
    query = np.asarray(inputs["query"], dtype=np.float32)
    key = np.asarray(inputs["key"], dtype=np.float32)
    attention_mask = np.asarray(inputs["attention_mask"], dtype=np.float32)
    target_tokens = np.asarray(inputs["target_tokens"]).astype(np.int64)
    brain_w = np.asarray(inputs["brain_w"], dtype=np.float32)
    storage_w1 = np.asarray(inputs["storage_w1"], dtype=np.float32)
    storage_b1 = np.asarray(inputs["storage_b1"], dtype=np.float32)
    storage_w2 = np.asarray(inputs["storage_w2"], dtype=np.float32)
    storage_b2 = np.asarray(inputs["storage_b2"], dtype=np.float32)
    sim_w = np.asarray(inputs["sim_w"], dtype=np.float32)
    sim_b = np.asarray(inputs["sim_b"], dtype=np.float32)
    memory_keys = np.asarray(inputs["memory_keys"], dtype=np.float32)
    memory_values = np.asarray(inputs["memory_values"], dtype=np.float32)
    memory_lengths = np.asarray(inputs["memory_lengths"])

    qflat = np.ascontiguousarray(query.reshape(TOK, D))
    qT = np.ascontiguousarray(qflat.T)                      # [D, TOK] f32
    tflat = target_tokens.reshape(-1)

    qt16_np = qT.astype(np.float16).reshape(4, 128, TOK)
    qtf_np = qT.reshape(4, 128, TOK)
    w1t_np = np.ascontiguousarray(storage_w1.T).reshape(4, 128, D)
    w2t_np = np.ascontiguousarray(storage_w2.T).reshape(4, 128, D)
    b1_np = np.ascontiguousarray(storage_b1[None, :])
    b2_np = np.ascontiguousarray(storage_b2[None, :])
    ones_np = np.ones((1, 128), np.float32)
    w16 = brain_w.astype(np.float16)                        # [V, D]

    in_maps = []
    for c in range(NCORES):
        c0 = c * VSH
        wt = w16[c0:c0 + VSH].T.reshape(4, 128, NV, VT)     # [D,VSH] -> chunks
        wt = np.ascontiguousarray(wt.transpose(2, 0, 1, 3))  # [NV,4,128,VT]
        r0 = c * SSH
        qts_np = np.ascontiguousarray(np.concatenate(
            [qT[:, r0:r0 + SSH], qT[:, S + r0:S + r0 + SSH]], axis=1)
        ).reshape(4, 128, 2 * SSH)
        t0 = c * RSH
        mxt_np = np.ascontiguousarray(np.concatenate(
            [qT[:, t0:t0 + RSH], np.zeros((D, NR - RSH), np.float32)], axis=1)
        ).reshape(4, 128, NR)
        in_maps.append(dict(
            wt16=wt, qt16=qt16_np, qtf=qtf_np, qts=qts_np, mxt=mxt_np,
            w1t=w1t_np, w2t=w2t_np, b1r=b1_np, b2r=b2_np, onesr=ones_np,
            qrows=np.ascontiguousarray(qflat[t0:t0 + RSH]),
            grows=np.ascontiguousarray(brain_w[tflat[t0:t0 + RSH]]),
        ))

    trace = bool(os.environ.get("KBENCH_TRACE"))
    res = run_bass_kernel_spmd(_get_nc(), in_maps, list(range(NCORES)),
                               trace=trace)
    LAST_RESULTS = res
    dev = res.results

    # ---- combine: surprise ----
    se = np.zeros(TOK, np.float64)
    tl = np.zeros(TOK, np.float64)
    for c in range(NCORES):
        se += dev[c]["se_o"].astype(np.float64).T.reshape(-1)
        tl[c * RSH:(c + 1) * RSH] = dev[c]["tl_o"][:, 0]
    surprise = (np.log(se) - tl).reshape(B, S)

    thr = _windowed_threshold(surprise)
    boundaries = (surprise > thr).astype(np.float64)

    sims = np.zeros((B, S, S), np.float32)
    for c in range(NCORES):
        sims[:, c * SSH:(c + 1) * SSH, :] = dev[c]["sims_o"]

    refined = _refine(sims, boundaries)

    # ---- event packing into memory slots ----
    labels = np.cumsum(refined, axis=1).astype(np.int64)
    events = []                                    # rank order: (b, s0, cnt)
    for b in range(B):
        lab = labels[b]
        starts = np.concatenate([[0], np.flatnonzero(np.diff(lab)) + 1])
        counts = np.diff(np.concatenate([starts, [S]]))
        for s0, cnt in zip(starts, counts):
            events.append((b, int(s0), int(cnt)))
    written = {}                                   # slot -> (b, s0, cnt)
    for rank, ev in enumerate(events):
        written[rank % NUM_MEM] = ev               # last write wins on wrap

    m_lens = memory_lengths.astype(np.int64).copy()
    valid = m_lens > 0
    key_vec = memory_keys.astype(np.float64).copy()
    for slot, (b, s0, cnt) in written.items():
        valid[slot] = True
        key_vec[slot] = qflat[b * S + s0]

    # ---- retrieval ----
    q64 = qflat.astype(np.float64).reshape(B, S, D)
    qm = q64.mean(1) @ sim_w.astype(np.float64).T + sim_b.astype(np.float64)
    scores = qm @ key_vec.T                        # [B, NUM_MEM]
    scores = np.where(valid[None, :], scores, -1e9)
    top_i = np.stack([_topk_idx(scores[b], K_SIM) for b in range(B)])
    rec = np.where(valid, np.arange(NUM_MEM), -1)
    cont_i = _topk_idx(rec.astype(np.float64), K_CONT)

    # ---- gather storage() rows ----
    Y = np.zeros((TOK, D), np.float32)
    for c in range(NCORES):
        Y[c * RSH:(c + 1) * RSH] = dev[c]["y_o"][:RSH]
    z_row = dev[0]["y_o"][RSH]

    def host_storage(x):
        h = np.maximum(x @ storage_w1.T + storage_b1, 0.0)
        return h @ storage_w2.T + storage_b2

    def slot_rows(slot):
        out = np.empty((MEM_LEN, D), np.float32)
        if slot in written:
            b, s0, cnt = written[slot]
            n = min(cnt, MEM_LEN)
            out[:n] = Y[b * S + s0:b * S + s0 + n]
            out[n:] = z_row
        else:
            mv = memory_values[slot]
            nz = np.any(mv != 0, axis=1)
            out[:] = z_row
            if nz.any():
                out[nz] = host_storage(mv[nz]).astype(np.float32)
        return out

    sim_part = np.stack([
        np.concatenate([slot_rows(s) for s in top_i[b]], axis=0)
        for b in range(B)])                        # [B, 128, D]
    cont_part = np.concatenate([slot_rows(s) for s in cont_i], axis=0)  # [64,D]

    context = np.concatenate(
        [sim_part, np.broadcast_to(cont_part[None], (B, K_CONT * MEM_LEN, D)),
         key], axis=1).astype(np.float32)
    ext_mask = np.concatenate(
        [np.ones((B, (K_SIM + K_CONT) * MEM_LEN), attention_mask.dtype),
         attention_mask], axis=1)

    return query, context, context, ext_mask


# revision 9
# speedup vs baseline: 1.3123x; 1.3123x over previous
"""Trainium2 Bass kernel for the PraxisMemory scatter_memory problem.

Strategy (8 NeuronCores, SPMD single launch):
  - Vocab-sharded fp16 logits GEMM (brain_w.T shard [512,4000] per core) with
    on-device exp + accumulation -> per-core partial sum-exp per token.
    Target-logit dot product kept in exact f32 on the vector engine.
  - Token-similarity matrix (sims) row-sharded across cores via float32r matmuls.
  - Storage MLP applied to every query token row (row-sharded, 64 rows + zero
    rows per core) so the data-dependent retrieval later is a pure gather.
  - Host (numpy, f64) does the tiny decision logic: logsumexp combine, windowed
    threshold, boundary refinement via prefix-sum modularity/conductance deltas,
    event packing into memory slots, top-k retrieval, and final concat.
"""

import os
import numpy as np

import concourse.bacc as bacc
import concourse.mybir as mybir
from concourse.tile import TileContext
from concourse.bass_utils import run_bass_kernel_spmd

F32 = mybir.dt.float32
F32R = mybir.dt.float32r
F16 = mybir.dt.float16
AF = mybir.ActivationFunctionType
AX = mybir.AxisListType

B, S, D, V = 2, 256, 512, 32000
W_WIN, GAMMA = 20, 2.0
MEM_LEN, NUM_MEM, K_SIM, K_CONT = 16, 256, 8, 4
NCORES = 8
VSH = V // NCORES           # 4000 vocab per core
NV, VT = 8, 500             # vocab tiles per core x tile width
TOK = B * S                 # 512 tokens
RSH = TOK // NCORES         # 64 token rows per core (MLP/tgt shard)
SSH = S // NCORES           # 32 sims rows per batch per core
NR = RSH + 4                # MLP rows per core: 64 query rows + 4 zero rows

_NC = None
LAST_RESULTS = None


def _build_nc():
    nc = bacc.Bacc("TRN2", target_bir_lowering=False, debug=False)

    # inputs (consolidated to minimize DMA instruction count)
    wt16 = nc.dram_tensor("wt16", [NV, 4, 128, VT], F16, kind="ExternalInput")
    qt16 = nc.dram_tensor("qt16", [4, 128, TOK], F16, kind="ExternalInput")
    qtf = nc.dram_tensor("qtf", [4, 128, TOK], F32R, kind="ExternalInput")
    qts = nc.dram_tensor("qts", [4, 128, 2 * SSH], F32R, kind="ExternalInput")
    mxt = nc.dram_tensor("mxt", [4, 128, NR], F32R, kind="ExternalInput")
    w1t = nc.dram_tensor("w1t", [4, 128, D], F32R, kind="ExternalInput")
    w2t = nc.dram_tensor("w2t", [4, 128, D], F32R, kind="ExternalInput")
    bb = nc.dram_tensor("bb", [1, 2 * D + 128], F32R, kind="ExternalInput")
    qg = nc.dram_tensor("qg", [RSH, 2 * D], F32, kind="ExternalInput")

    # outputs
    setl_o = nc.dram_tensor("setl_o", [128, 5], F32, kind="ExternalOutput")
    sims_o = nc.dram_tensor("sims_o", [B, SSH, S], F32, kind="ExternalOutput")
    y_o = nc.dram_tensor("y_o", [NR, D], F32, kind="ExternalOutput")

    with TileContext(nc) as tc:
        with tc.tile_pool(name="const", bufs=1) as cpool, \
             tc.tile_pool(name="wts", bufs=2) as wpool, \
             tc.tile_pool(name="work", bufs=3) as work, \
             tc.tile_pool(name="pslg", bufs=2, space="PSUM") as pslg, \
             tc.tile_pool(name="pssm", bufs=1, space="PSUM") as pssm:

            # ---- logits-critical loads first (HWDGE ring is FIFO) ----
            qt16_sb = cpool.tile([128, 4 * TOK], F16, tag="qt16")
            nc.sync.dma_start(
                qt16_sb[:].rearrange("p (k t) -> p k t", k=4),
                qt16[:].rearrange("k p t -> p k t"))
            wt_sb = []
            for n in range(NV):
                wt = wpool.tile([128, 4 * VT], F16, tag=f"wt_{n % 2}",
                                name=f"wt_{n}")
                nc.sync.dma_start(
                    wt[:].rearrange("p (k t) -> p k t", k=4),
                    wt16[n].rearrange("k p t -> p k t"))
                wt_sb.append(wt)
                if n == 0:
                    # small loads ride between the big weight transfers
                    qg_sb = cpool.tile([RSH, 2 * D], F32, tag="qg")
                    nc.sync.dma_start(qg_sb[:], qg[:])
                if n == 1:
                    qtf_sb = cpool.tile([128, 4 * TOK], F32R, tag="qtf")
                    nc.sync.dma_start(
                        qtf_sb[:].rearrange("p (k t) -> p k t", k=4),
                        qtf[:].rearrange("k p t -> p k t"))
                    qts_sb = cpool.tile([128, 4 * 2 * SSH], F32R, tag="qts")
                    nc.sync.dma_start(
                        qts_sb[:].rearrange("p (k t) -> p k t", k=4),
                        qts[:].rearrange("k p t -> p k t"))
                if n == 2:
                    mxt_sb = cpool.tile([128, 4 * NR], F32R, tag="mxt")
                    nc.sync.dma_start(
                        mxt_sb[:].rearrange("p (k t) -> p k t", k=4),
                        mxt[:].rearrange("k p t -> p k t"))
                    w1t_sb = cpool.tile([128, 4 * D], F32R, tag="w1t")
                    nc.sync.dma_start(
                        w1t_sb[:].rearrange("p (k t) -> p k t", k=4),
                        w1t[:].rearrange("k p t -> p k t"))
                if n == 3:
                    w2t_sb = cpool.tile([128, 4 * D], F32R, tag="w2t")
                    nc.sync.dma_start(
                        w2t_sb[:].rearrange("p (k t) -> p k t", k=4),
                        w2t[:].rearrange("k p t -> p k t"))
                    bb_sb = cpool.tile([1, 2 * D + 128], F32R, tag="bb")
                    nc.sync.dma_start(bb_sb[:], bb[:])

            # ---- logits GEMM (fp16) + exp-accumulate ----
            separts = [cpool.tile([128, NV], F32, tag=f"sep_{m}",
                                  name=f"sep_{m}") for m in range(4)]
            for n in range(NV):
                for m in range(4):
                    ps = pslg.tile([128, VT], F32, tag="lg")
                    for k in range(4):
                        nc.tensor.matmul(
                            ps[:],
                            lhsT=qt16_sb[:, k * TOK + m * 128:
                                         k * TOK + (m + 1) * 128],
                            rhs=wt_sb[n][:, k * VT:(k + 1) * VT],
                            start=(k == 0), stop=(k == 3))
                    et = work.tile([128, VT], F32, tag="exp")
                    nc.scalar.activation(et[:], ps[:], AF.Exp,
                                         accum_out=separts[m][:, n:n + 1])

            # combined [128,5] output: cols 0-3 sum-exp per token tile,
            # col 4 rows 0-63 target logits
            setl_sb = cpool.tile([128, 5], F32, tag="setl")
            nc.gpsimd.memset(setl_sb[:, 4:5], 0.0)
            for m in range(4):
                nc.vector.reduce_sum(setl_sb[:, m:m + 1], separts[m][:],
                                     axis=AX.X)

            # ---- target-logit dot (exact f32 on DVE) ----
            prod = work.tile([RSH, D], F32, tag="prod")
            nc.vector.tensor_mul(prod[:], qg_sb[:, :D], qg_sb[:, D:])
            nc.vector.reduce_sum(setl_sb[:RSH, 4:5], prod[:], axis=AX.X)
            nc.sync.dma_start(setl_o[:], setl_sb[:])

            # ---- sims row-shard (f32r) ----
            for b in range(B):
                pss = pssm.tile([SSH, S], F32, tag="sims")
                for k in range(4):
                    nc.tensor.matmul(
                        pss[:],
                        lhsT=qts_sb[:, k * 2 * SSH + b * SSH:
                                    k * 2 * SSH + (b + 1) * SSH],
                        rhs=qtf_sb[:, k * TOK + b * S:k * TOK + (b + 1) * S],
                        start=(k == 0), stop=(k == 3))
                so = work.tile([SSH, S], F32, tag="so")
                nc.vector.tensor_copy(so[:], pss[:])
                nc.sync.dma_start(sims_o[b], so[:])

            # ---- storage MLP on 64 query rows + zero rows ----
            ht_sb = []
            for e in range(4):
                psh = pssm.tile([128, NR], F32, tag="psh")
                for k in range(4):
                    nc.tensor.matmul(
                        psh[:],
                        lhsT=w1t_sb[:, k * D + e * 128:k * D + (e + 1) * 128],
                        rhs=mxt_sb[:, k * NR:(k + 1) * NR],
                        start=(k == 0), stop=False)
                nc.tensor.matmul(psh[:], lhsT=bb_sb[:, e * 128:(e + 1) * 128],
                                 rhs=bb_sb[:, 2 * D:2 * D + NR],
                                 start=False, stop=True)
                hf = work.tile([128, NR], F32, tag="hf")
                nc.scalar.activation(hf[:], psh[:], AF.Relu)
                ht = cpool.tile([128, NR], F32R, tag=f"ht_{e}", name=f"ht_{e}")
                nc.vector.tensor_copy(ht[:], hf[:])
                ht_sb.append(ht)
            psy = pssm.tile([NR, D], F32, tag="psy")
            for e in range(4):
                nc.tensor.matmul(psy[:], lhsT=ht_sb[e][:],
                                 rhs=w2t_sb[:, e * D:(e + 1) * D],
                                 start=(e == 0), stop=False)
            nc.tensor.matmul(psy[:], lhsT=bb_sb[:, 2 * D:2 * D + NR],
                             rhs=bb_sb[:, D:2 * D], start=False, stop=True)
            y_sb = work.tile([NR, D], F32, tag="y")
            nc.vector.tensor_copy(y_sb[:], psy[:])
            nc.sync.dma_start(y_o[:], y_sb[:])

    nc.compile()
    return nc


def _get_nc():
    global _NC
    if _NC is None:
        _NC = _build_nc()
    return _NC


# ---------------------------------------------------------------- host logic

def _windowed_threshold(sur):
    wv = np.lib.stride_tricks.sliding_window_view(sur, W_WIN, axis=1)
    t = wv.mean(-1) + GAMMA * wv.std(-1, ddof=1)
    return np.concatenate([np.repeat(t[:, :1], W_WIN - 1, axis=1), t], axis=1)


def _refine(sims, bnd):
    """Candidate boundary-removal decisions via analytic merge deltas (f64)."""
    refined = bnd.copy()
    for b in range(B):
        sim = sims[b].astype(np.float64)
        bd = bnd[b]
        deg = sim.sum(1)
        tot = deg.sum()
        com = np.cumsum(bd).astype(np.int64)
        nlast = int(com[-1])
        mask = (com[:, None] == com[None, :])
        expd = deg[:, None] * deg[None, :] / (tot + 1e-10)
        curQ = ((sim - expd) * mask).sum() / (tot + 1e-10)
        nmax = nlast + 1
        vol = np.zeros(nmax)
        intra = np.zeros(nmax)
        for c in np.unique(com):
            idx = com == c
            vol[c] = deg[idx].sum()
            intra[c] = sim[np.ix_(idx, idx)].sum()
        cuts = vol - intra
        minv = np.minimum(vol, tot - vol)
        cond = np.where(vol > 0, cuts / (minv + 1e-10), 0.0)
        cond_sum = cond.sum()
        curC = cond_sum / (nlast + 1.0)
        for t in np.flatnonzero(bd > 0):
            c = int(com[t])
            A = com == (c - 1)
            Bm = com == c
            S_AB = sim[np.ix_(A, Bm)].sum()
            degA = deg[A].sum()
            degB = deg[Bm].sum()
            newQ = curQ + 2.0 * (S_AB - degA * degB / (tot + 1e-10)) / (tot + 1e-10)
            volM = vol[c - 1] + vol[c]
            intraM = intra[c - 1] + intra[c] + 2.0 * S_AB
            condM = (volM - intraM) / (np.minimum(volM, tot - volM) + 1e-10)
            newC = (cond_sum - cond[c - 1] - cond[c] + condM) / float(nlast)
            if (newQ > curQ) or (newC < curC):
                refined[b, t] = 0.0
    return refined


def _topk_idx(values, k):
    """jax.lax.top_k semantics: descending values, ties -> lowest index."""
    n = len(values)
    order = np.lexsort((np.arange(n), -values))
    return order[:k]


def kernel(**inputs):
    global LAST_RESULTS
    query = np.asarray(inputs["query"], dtype=np.float32)
    key = np.asarray(inputs["key"], dtype=np.float32)
    attention_mask = np.asarray(inputs["attention_mask"], dtype=np.float32)
    target_tokens = np.asarray(inputs["target_tokens"]).astype(np.int64)
    brain_w = np.asarray(inputs["brain_w"], dtype=np.float32)
    storage_w1 = np.asarray(inputs["storage_w1"], dtype=np.float32)
    storage_b1 = np.asarray(inputs["storage_b1"], dtype=np.float32)
    storage_w2 = np.asarray(inputs["storage_w2"], dtype=np.float32)
    storage_b2 = np.asarray(inputs["storage_b2"], dtype=np.float32)
    sim_w = np.asarray(inputs["sim_w"], dtype=np.float32)
    sim_b = np.asarray(inputs["sim_b"], dtype=np.float32)
    memory_keys = np.asarray(inputs["memory_keys"], dtype=np.float32)
    memory_values = np.asarray(inputs["memory_values"], dtype=np.float32)
    memory_lengths = np.asarray(inputs["memory_lengths"])

    qflat = np.ascontiguousarray(query.reshape(TOK, D))
    qT = np.ascontiguousarray(qflat.T)                      # [D, TOK] f32
    tflat = target_tokens.reshape(-1)

    qt16_np = qT.astype(np.float16).reshape(4, 128, TOK)
    qtf_np = qT.reshape(4, 128, TOK)
    w1t_np = np.ascontiguousarray(storage_w1.T).reshape(4, 128, D)
    w2t_np = np.ascontiguousarray(storage_w2.T).reshape(4, 128, D)
    bb_np = np.concatenate(
        [storage_b1, storage_b2, np.ones(128, np.float32)])[None, :]
    w16 = brain_w.astype(np.float16)                        # [V, D]

    in_maps = []
    for c in range(NCORES):
        c0 = c * VSH
        wt = w16[c0:c0 + VSH].T.reshape(4, 128, NV, VT)     # [D,VSH] -> chunks
        wt = np.ascontiguousarray(wt.transpose(2, 0, 1, 3))  # [NV,4,128,VT]
        r0 = c * SSH
        qts_np = np.ascontiguousarray(np.concatenate(
            [qT[:, r0:r0 + SSH], qT[:, S + r0:S + r0 + SSH]], axis=1)
        ).reshape(4, 128, 2 * SSH)
        t0 = c * RSH
        mxt_np = np.ascontiguousarray(np.concatenate(
            [qT[:, t0:t0 + RSH], np.zeros((D, NR - RSH), np.float32)], axis=1)
        ).reshape(4, 128, NR)
        qg_np = np.concatenate(
            [qflat[t0:t0 + RSH], brain_w[tflat[t0:t0 + RSH]]], axis=1)
        in_maps.append(dict(
            wt16=wt, qt16=qt16_np, qtf=qtf_np, qts=qts_np, mxt=mxt_np,
            w1t=w1t_np, w2t=w2t_np, bb=bb_np,
            qg=np.ascontiguousarray(qg_np),
        ))

    trace = bool(os.environ.get("KBENCH_TRACE"))
    res = run_bass_kernel_spmd(_get_nc(), in_maps, list(range(NCORES)),
                               trace=trace)
    LAST_RESULTS = res
    dev = res.results

    # ---- combine: surprise ----
    se = np.zeros(TOK, np.float64)
    tl = np.zeros(TOK, np.float64)
    for c in range(NCORES):
        se += dev[c]["setl_o"][:, :4].astype(np.float64).T.reshape(-1)
        tl[c * RSH:(c + 1) * RSH] = dev[c]["setl_o"][:RSH, 4]
    surprise = (np.log(se) - tl).reshape(B, S)

    thr = _windowed_threshold(surprise)
    boundaries = (surprise > thr).astype(np.float64)

    sims = np.zeros((B, S, S), np.float32)
    for c in range(NCORES):
        sims[:, c * SSH:(c + 1) * SSH, :] = dev[c]["sims_o"]

    refined = _refine(sims, boundaries)

    # ---- event packing into memory slots ----
    labels = np.cumsum(refined, axis=1).astype(np.int64)
    events = []                                    # rank order: (b, s0, cnt)
    for b in range(B):
        lab = labels[b]
        starts = np.concatenate([[0], np.flatnonzero(np.diff(lab)) + 1])
        counts = np.diff(np.concatenate([starts, [S]]))
        for s0, cnt in zip(starts, counts):
            events.append((b, int(s0), int(cnt)))
    written = {}                                   # slot -> (b, s0, cnt)
    for rank, ev in enumerate(events):
        written[rank % NUM_MEM] = ev               # last write wins on wrap

    m_lens = memory_lengths.astype(np.int64).copy()
    valid = m_lens > 0
    key_vec = memory_keys.astype(np.float64).copy()
    for slot, (b, s0, cnt) in written.items():
        valid[slot] = True
        key_vec[slot] = qflat[b * S + s0]

    # ---- retrieval ----
    q64 = qflat.astype(np.float64).reshape(B, S, D)
    qm = q64.mean(1) @ sim_w.astype(np.float64).T + sim_b.astype(np.float64)
    scores = qm @ key_vec.T                        # [B, NUM_MEM]
    scores = np.where(valid[None, :], scores, -1e9)
    top_i = np.stack([_topk_idx(scores[b], K_SIM) for b in range(B)])
    rec = np.where(valid, np.arange(NUM_MEM), -1)
    cont_i = _topk_idx(rec.astype(np.float64), K_CONT)

    # ---- gather storage() rows ----
    Y = np.zeros((TOK, D), np.float32)
    for c in range(NCORES):
        Y[c * RSH:(c + 1) * RSH] = dev[c]["y_o"][:RSH]
    z_row = dev[0]["y_o"][RSH]

    def host_storage(x):
        h = np.maximum(x @ storage_w1.T + storage_b1, 0.0)
        return h @ storage_w2.T + storage_b2

    def slot_rows(slot):
        out = np.empty((MEM_LEN, D), np.float32)
        if slot in written:
            b, s0, cnt = written[slot]
            n = min(cnt, MEM_LEN)
            out[:n] = Y[b * S + s0:b * S + s0 + n]
            out[n:] = z_row
        else:
            mv = memory_values[slot]
            nz = np.any(mv != 0, axis=1)
            out[:] = z_row
            if nz.any():
                out[nz] = host_storage(mv[nz]).astype(np.float32)
        return out

    sim_part = np.stack([
        np.concatenate([slot_rows(s) for s in top_i[b]], axis=0)
        for b in range(B)])                        # [B, 128, D]
    cont_part = np.concatenate([slot_rows(s) for s in cont_i], axis=0)  # [64,D]

    context = np.concatenate(
        [sim_part, np.broadcast_to(cont_part[None], (B, K_CONT * MEM_LEN, D)),
         key], axis=1).astype(np.float32)
    ext_mask = np.concatenate(
        [np.ones((B, (K_SIM + K_CONT) * MEM_LEN), attention_mask.dtype),
         attention_mask], axis=1)

    return query, context, context, ext_mask


# revision 10
# speedup vs baseline: 1.4263x; 1.0869x over previous
"""Trainium2 Bass kernel for the PraxisMemory scatter_memory problem.

Strategy (8 NeuronCores, SPMD single launch):
  - Vocab-sharded fp16 logits GEMM (brain_w.T shard [512,4000] per core) with
    on-device exp + accumulation -> per-core partial sum-exp per token.
    Target-logit dot product kept in exact f32 on the vector engine.
  - Token-similarity matrix (sims) row-sharded across cores via float32r matmuls.
  - Storage MLP applied to every query token row (row-sharded, 64 rows + zero
    rows per core) so the data-dependent retrieval later is a pure gather.
  - Host (numpy, f64) does the tiny decision logic: logsumexp combine, windowed
    threshold, boundary refinement via prefix-sum modularity/conductance deltas,
    event packing into memory slots, top-k retrieval, and final concat.
"""

import os
import numpy as np

import concourse.bacc as bacc
import concourse.mybir as mybir
from concourse.tile import TileContext
from concourse.bass_utils import run_bass_kernel_spmd

F32 = mybir.dt.float32
F32R = mybir.dt.float32r
F16 = mybir.dt.float16
F8 = mybir.dt.float8e4
AF = mybir.ActivationFunctionType
AX = mybir.AxisListType

B, S, D, V = 2, 256, 512, 32000
W_WIN, GAMMA = 20, 2.0
MEM_LEN, NUM_MEM, K_SIM, K_CONT = 16, 256, 8, 4
NCORES = 8
VSH = V // NCORES           # 4000 vocab per core
NV, VT = 8, 512             # vocab tiles per core x tile width (zero-padded)
VPAD = NV * VT - VSH        # 96 zero-padded vocab columns per core
TOK = B * S                 # 512 tokens
RSH = TOK // NCORES         # 64 token rows per core (MLP/tgt shard)
SSH = S // NCORES           # 32 sims rows per batch per core
NR = RSH + 4                # MLP rows per core: 64 query rows + 4 zero rows

_NC = None
LAST_RESULTS = None


def _build_nc():
    nc = bacc.Bacc("TRN2", target_bir_lowering=False, debug=False)

    # inputs (consolidated to minimize DMA instruction count)
    wt8 = nc.dram_tensor("wt8", [NV, 2, 128, 2 * VT], F8, kind="ExternalInput")
    qt8 = nc.dram_tensor("qt8", [2, 128, 2 * TOK], F8, kind="ExternalInput")
    qtf = nc.dram_tensor("qtf", [4, 128, TOK], F32R, kind="ExternalInput")
    qts = nc.dram_tensor("qts", [4, 128, 2 * SSH], F32R, kind="ExternalInput")
    mxt = nc.dram_tensor("mxt", [4, 128, NR], F32R, kind="ExternalInput")
    w1t = nc.dram_tensor("w1t", [4, 128, D], F32R, kind="ExternalInput")
    w2t = nc.dram_tensor("w2t", [4, 128, D], F32R, kind="ExternalInput")
    bb = nc.dram_tensor("bb", [1, 2 * D + 128], F32R, kind="ExternalInput")
    qg = nc.dram_tensor("qg", [RSH, 2 * D], F32, kind="ExternalInput")

    # outputs
    setl_o = nc.dram_tensor("setl_o", [128, 5], F32, kind="ExternalOutput")
    sims_o = nc.dram_tensor("sims_o", [B, SSH, S], F32, kind="ExternalOutput")
    y_o = nc.dram_tensor("y_o", [NR, D], F32, kind="ExternalOutput")

    with TileContext(nc) as tc:
        with tc.tile_pool(name="const", bufs=1) as cpool, \
             tc.tile_pool(name="wts", bufs=2) as wpool, \
             tc.tile_pool(name="work", bufs=3) as work, \
             tc.tile_pool(name="psum", bufs=2, space="PSUM") as psp:

            # ---- logits-critical loads first (HWDGE ring is FIFO) ----
            qt8_sb = cpool.tile([128, 2 * 2 * TOK], F8, tag="qt8")
            nc.sync.dma_start(
                qt8_sb[:].rearrange("p (k t) -> p k t", k=2),
                qt8[:].rearrange("k p t -> p k t"))
            wt_sb = []
            for n in range(NV):
                wt = wpool.tile([128, 2 * 2 * VT], F8, tag=f"wt_{n % 2}",
                                name=f"wt_{n}")
                nc.sync.dma_start(
                    wt[:].rearrange("p (k t) -> p k t", k=2),
                    wt8[n].rearrange("k p t -> p k t"))
                wt_sb.append(wt)
                if n == 0:
                    # small loads ride between the big weight transfers
                    qg_sb = cpool.tile([RSH, 2 * D], F32, tag="qg")
                    nc.sync.dma_start(qg_sb[:], qg[:])
                if n == 1:
                    qtf_sb = cpool.tile([128, 4 * TOK], F32R, tag="qtf")
                    nc.sync.dma_start(
                        qtf_sb[:].rearrange("p (k t) -> p k t", k=4),
                        qtf[:].rearrange("k p t -> p k t"))
                    qts_sb = cpool.tile([128, 4 * 2 * SSH], F32R, tag="qts")
                    nc.sync.dma_start(
                        qts_sb[:].rearrange("p (k t) -> p k t", k=4),
                        qts[:].rearrange("k p t -> p k t"))
                if n == 2:
                    mxt_sb = cpool.tile([128, 4 * NR], F32R, tag="mxt")
                    nc.sync.dma_start(
                        mxt_sb[:].rearrange("p (k t) -> p k t", k=4),
                        mxt[:].rearrange("k p t -> p k t"))
                    w1t_sb = cpool.tile([128, 4 * D], F32R, tag="w1t")
                    nc.sync.dma_start(
                        w1t_sb[:].rearrange("p (k t) -> p k t", k=4),
                        w1t[:].rearrange("k p t -> p k t"))
                if n == 3:
                    w2t_sb = cpool.tile([128, 4 * D], F32R, tag="w2t")
                    nc.sync.dma_start(
                        w2t_sb[:].rearrange("p (k t) -> p k t", k=4),
                        w2t[:].rearrange("k p t -> p k t"))
                    bb_sb = cpool.tile([1, 2 * D + 128], F32R, tag="bb")
                    nc.sync.dma_start(bb_sb[:], bb[:])

            # ---- logits GEMM (fp8 DoubleRow) + exp-accumulate ----
            separts = [cpool.tile([128, 2], F32, tag=f"sep_{m}",
                                  name=f"sep_{m}") for m in range(4)]
            for g in range(2):
                for m in range(4):
                    ps = psp.tile([128, 4 * VT], F32, tag="lg")
                    for h in range(4):
                        n = g * 4 + h
                        for kk in range(2):
                            lv = qt8_sb[:, kk * 2 * TOK:(kk + 1) * 2 * TOK]
                            lv = lv.rearrange("p (two t) -> p two t", two=2)
                            rv = wt_sb[n][:, kk * 2 * VT:(kk + 1) * 2 * VT]
                            rv = rv.rearrange("p (two v) -> p two v", two=2)
                            nc.tensor.matmul(
                                ps[:, h * VT:(h + 1) * VT],
                                lhsT=lv[:, :, m * 128:(m + 1) * 128],
                                rhs=rv,
                                start=(kk == 0), stop=(kk == 1),
                                perf_mode=mybir.MatmulPerfMode.DoubleRow)
                    et = work.tile([128, 4 * VT], F32, tag="exp")
                    nc.scalar.activation(et[:], ps[:], AF.Exp,
                                         accum_out=separts[m][:, g:g + 1])

            # combined [128,5] output: cols 0-3 sum-exp per token tile,
            # col 4 rows 0-63 target logits
            setl_sb = cpool.tile([128, 5], F32, tag="setl")
            nc.gpsimd.memset(setl_sb[:, 4:5], 0.0)
            for m in range(4):
                nc.vector.reduce_sum(setl_sb[:, m:m + 1], separts[m][:],
                                     axis=AX.X)

            # ---- target-logit dot (exact f32 on DVE) ----
            prod = work.tile([RSH, D], F32, tag="prod")
            nc.vector.tensor_mul(prod[:], qg_sb[:, :D], qg_sb[:, D:])
            nc.vector.reduce_sum(setl_sb[:RSH, 4:5], prod[:], axis=AX.X)
            nc.sync.dma_start(setl_o[:], setl_sb[:])

            # ---- sims row-shard (f32r) ----
            for b in range(B):
                pss = psp.tile([SSH, S], F32, tag="lg", name="pss")
                for k in range(4):
                    nc.tensor.matmul(
                        pss[:],
                        lhsT=qts_sb[:, k * 2 * SSH + b * SSH:
                                    k * 2 * SSH + (b + 1) * SSH],
                        rhs=qtf_sb[:, k * TOK + b * S:k * TOK + (b + 1) * S],
                        start=(k == 0), stop=(k == 3))
                so = work.tile([SSH, S], F32, tag="so")
                nc.vector.tensor_copy(so[:], pss[:])
                nc.sync.dma_start(sims_o[b], so[:])

            # ---- storage MLP on 64 query rows + zero rows ----
            ht_sb = []
            for e in range(4):
                psh = psp.tile([128, NR], F32, tag="lg", name="psh")
                for k in range(4):
                    nc.tensor.matmul(
                        psh[:],
                        lhsT=w1t_sb[:, k * D + e * 128:k * D + (e + 1) * 128],
                        rhs=mxt_sb[:, k * NR:(k + 1) * NR],
                        start=(k == 0), stop=False)
                nc.tensor.matmul(psh[:], lhsT=bb_sb[:, e * 128:(e + 1) * 128],
                                 rhs=bb_sb[:, 2 * D:2 * D + NR],
                                 start=False, stop=True)
                hf = work.tile([128, NR], F32, tag="hf")
                nc.scalar.activation(hf[:], psh[:], AF.Relu)
                ht = cpool.tile([128, NR], F32R, tag=f"ht_{e}", name=f"ht_{e}")
                nc.vector.tensor_copy(ht[:], hf[:])
                ht_sb.append(ht)
            psy = psp.tile([NR, D], F32, tag="lg", name="psy")
            for e in range(4):
                nc.tensor.matmul(psy[:], lhsT=ht_sb[e][:],
                                 rhs=w2t_sb[:, e * D:(e + 1) * D],
                                 start=(e == 0), stop=False)
            nc.tensor.matmul(psy[:], lhsT=bb_sb[:, 2 * D:2 * D + NR],
                             rhs=bb_sb[:, D:2 * D], start=False, stop=True)
            y_sb = work.tile([NR, D], F32, tag="y")
            nc.vector.tensor_copy(y_sb[:], psy[:])
            nc.sync.dma_start(y_o[:], y_sb[:])

    nc.compile()
    return nc


def _get_nc():
    global _NC
    if _NC is None:
        _NC = _build_nc()
    return _NC


# ---------------------------------------------------------------- host logic

def _windowed_threshold(sur):
    wv = np.lib.stride_tricks.sliding_window_view(sur, W_WIN, axis=1)
    t = wv.mean(-1) + GAMMA * wv.std(-1, ddof=1)
    return np.concatenate([np.repeat(t[:, :1], W_WIN - 1, axis=1), t], axis=1)


def _refine(sims, bnd):
    """Candidate boundary-removal decisions via analytic merge deltas (f64)."""
    refined = bnd.copy()
    for b in range(B):
        sim = sims[b].astype(np.float64)
        bd = bnd[b]
        deg = sim.sum(1)
        tot = deg.sum()
        com = np.cumsum(bd).astype(np.int64)
        nlast = int(com[-1])
        mask = (com[:, None] == com[None, :])
        expd = deg[:, None] * deg[None, :] / (tot + 1e-10)
        curQ = ((sim - expd) * mask).sum() / (tot + 1e-10)
        nmax = nlast + 1
        vol = np.zeros(nmax)
        intra = np.zeros(nmax)
        for c in np.unique(com):
            idx = com == c
            vol[c] = deg[idx].sum()
            intra[c] = sim[np.ix_(idx, idx)].sum()
        cuts = vol - intra
        minv = np.minimum(vol, tot - vol)
        cond = np.where(vol > 0, cuts / (minv + 1e-10), 0.0)
        cond_sum = cond.sum()
        curC = cond_sum / (nlast + 1.0)
        for t in np.flatnonzero(bd > 0):
            c = int(com[t])
            A = com == (c - 1)
            Bm = com == c
            S_AB = sim[np.ix_(A, Bm)].sum()
            degA = deg[A].sum()
            degB = deg[Bm].sum()
            newQ = curQ + 2.0 * (S_AB - degA * degB / (tot + 1e-10)) / (tot + 1e-10)
            volM = vol[c - 1] + vol[c]
            intraM = intra[c - 1] + intra[c] + 2.0 * S_AB
            condM = (volM - intraM) / (np.minimum(volM, tot - volM) + 1e-10)
            newC = (cond_sum - cond[c - 1] - cond[c] + condM) / float(nlast)
            if (newQ > curQ) or (newC < curC):
                refined[b, t] = 0.0
    return refined


def _topk_idx(values, k):
    """jax.lax.top_k semantics: descending values, ties -> lowest index."""
    n = len(values)
    order = np.lexsort((np.arange(n), -values))
    return order[:k]


def kernel(**inputs):
    global LAST_RESULTS
    query = np.asarray(inputs["query"], dtype=np.float32)
    key = np.asarray(inputs["key"], dtype=np.float32)
    attention_mask = np.asarray(inputs["attention_mask"], dtype=np.float32)
    target_tokens = np.asarray(inputs["target_tokens"]).astype(np.int64)
    brain_w = np.asarray(inputs["brain_w"], dtype=np.float32)
    storage_w1 = np.asarray(inputs["storage_w1"], dtype=np.float32)
    storage_b1 = np.asarray(inputs["storage_b1"], dtype=np.float32)
    storage_w2 = np.asarray(inputs["storage_w2"], dtype=np.float32)
    storage_b2 = np.asarray(inputs["storage_b2"], dtype=np.float32)
    sim_w = np.asarray(inputs["sim_w"], dtype=np.float32)
    sim_b = np.asarray(inputs["sim_b"], dtype=np.float32)
    memory_keys = np.asarray(inputs["memory_keys"], dtype=np.float32)
    memory_values = np.asarray(inputs["memory_values"], dtype=np.float32)
    memory_lengths = np.asarray(inputs["memory_lengths"])

    qflat = np.ascontiguousarray(query.reshape(TOK, D))
    qT = np.ascontiguousarray(qflat.T)                      # [D, TOK] f32
    tflat = target_tokens.reshape(-1)

    import ml_dtypes
    F8NP = ml_dtypes.float8_e4m3
    qt8_np = np.ascontiguousarray(
        qT.astype(F8NP).reshape(2, 2, 128, TOK).transpose(0, 2, 1, 3)
    ).reshape(2, 128, 2 * TOK)
    qtf_np = qT.reshape(4, 128, TOK)
    w1t_np = np.ascontiguousarray(storage_w1.T).reshape(4, 128, D)
    w2t_np = np.ascontiguousarray(storage_w2.T).reshape(4, 128, D)
    bb_np = np.concatenate(
        [storage_b1, storage_b2, np.ones(128, np.float32)])[None, :]
    w8tp = np.zeros((NCORES, D, NV * VT), F8NP)             # padded W.T shards
    wt8_all = brain_w.astype(F8NP)

    in_maps = []
    for c in range(NCORES):
        c0 = c * VSH
        w8tp[c][:, :VSH] = wt8_all[c0:c0 + VSH].T
        wt = np.ascontiguousarray(
            w8tp[c].reshape(2, 2, 128, NV, VT).transpose(3, 0, 2, 1, 4)
        ).reshape(NV, 2, 128, 2 * VT)
        r0 = c * SSH
        qts_np = np.ascontiguousarray(np.concatenate(
            [qT[:, r0:r0 + SSH], qT[:, S + r0:S + r0 + SSH]], axis=1)
        ).reshape(4, 128, 2 * SSH)
        t0 = c * RSH
        mxt_np = np.ascontiguousarray(np.concatenate(
            [qT[:, t0:t0 + RSH], np.zeros((D, NR - RSH), np.float32)], axis=1)
        ).reshape(4, 128, NR)
        qg_np = np.concatenate(
            [qflat[t0:t0 + RSH], brain_w[tflat[t0:t0 + RSH]]], axis=1)
        in_maps.append(dict(
            wt8=wt, qt8=qt8_np, qtf=qtf_np, qts=qts_np, mxt=mxt_np,
            w1t=w1t_np, w2t=w2t_np, bb=bb_np,
            qg=np.ascontiguousarray(qg_np),
        ))

    trace = bool(os.environ.get("KBENCH_TRACE"))
    res = run_bass_kernel_spmd(_get_nc(), in_maps, list(range(NCORES)),
                               trace=trace)
    LAST_RESULTS = res
    dev = res.results

    # ---- combine: surprise ----
    se = np.zeros(TOK, np.float64)
    tl = np.zeros(TOK, np.float64)
    for c in range(NCORES):
        se += dev[c]["setl_o"][:, :4].astype(np.float64).T.reshape(-1)
        tl[c * RSH:(c + 1) * RSH] = dev[c]["setl_o"][:RSH, 4]
    se -= float(NCORES * VPAD)        # exp(0)=1 per zero-padded vocab column
    surprise = (np.log(se) - tl).reshape(B, S)

    thr = _windowed_threshold(surprise)
    boundaries = (surprise > thr).astype(np.float64)

    sims = np.zeros((B, S, S), np.float32)
    for c in range(NCORES):
        sims[:, c * SSH:(c + 1) * SSH, :] = dev[c]["sims_o"]

    refined = _refine(sims, boundaries)

    # ---- event packing into memory slots ----
    labels = np.cumsum(refined, axis=1).astype(np.int64)
    events = []                                    # rank order: (b, s0, cnt)
    for b in range(B):
        lab = labels[b]
        starts = np.concatenate([[0], np.flatnonzero(np.diff(lab)) + 1])
        counts = np.diff(np.concatenate([starts, [S]]))
        for s0, cnt in zip(starts, counts):
            events.append((b, int(s0), int(cnt)))
    written = {}                                   # slot -> (b, s0, cnt)
    for rank, ev in enumerate(events):
        written[rank % NUM_MEM] = ev               # last write wins on wrap

    m_lens = memory_lengths.astype(np.int64).copy()
    valid = m_lens > 0
    key_vec = memory_keys.astype(np.float64).copy()
    for slot, (b, s0, cnt) in written.items():
        valid[slot] = True
        key_vec[slot] = qflat[b * S + s0]

    # ---- retrieval ----
    q64 = qflat.astype(np.float64).reshape(B, S, D)
    qm = q64.mean(1) @ sim_w.astype(np.float64).T + sim_b.astype(np.float64)
    scores = qm @ key_vec.T                        # [B, NUM_MEM]
    scores = np.where(valid[None, :], scores, -1e9)
    top_i = np.stack([_topk_idx(scores[b], K_SIM) for b in range(B)])
    rec = np.where(valid, np.arange(NUM_MEM), -1)
    cont_i = _topk_idx(rec.astype(np.float64), K_CONT)

    # ---- gather storage() rows ----
    Y = np.zeros((TOK, D), np.float32)
    for c in range(NCORES):
        Y[c * RSH:(c + 1) * RSH] = dev[c]["y_o"][:RSH]
    z_row = dev[0]["y_o"][RSH]

    def host_storage(x):
        h = np.maximum(x @ storage_w1.T + storage_b1, 0.0)
        return h @ storage_w2.T + storage_b2

    def slot_rows(slot):
        out = np.empty((MEM_LEN, D), np.float32)
        if slot in written:
            b, s0, cnt = written[slot]
            n = min(cnt, MEM_LEN)
            out[:n] = Y[b * S + s0:b * S + s0 + n]
            out[n:] = z_row
        else:
            mv = memory_values[slot]
            nz = np.any(mv != 0, axis=1)
            out[:] = z_row
            if nz.any():
                out[nz] = host_storage(mv[nz]).astype(np.float32)
        return out

    sim_part = np.stack([
        np.concatenate([slot_rows(s) for s in top_i[b]], axis=0)
        for b in range(B)])                        # [B, 128, D]
    cont_part = np.concatenate([slot_rows(s) for s in cont_i], axis=0)  # [64,D]

    context = np.concatenate(
        [sim_part, np.broadcast_to(cont_part[None], (B, K_CONT * MEM_LEN, D)),
         key], axis=1).astype(np.float32)
    ext_mask = np.concatenate(
        [np.ones((B, (K_SIM + K_CONT) * MEM_LEN), attention_mask.dtype),
         attention_mask], axis=1)

    return query, context, context, ext_mask


# revision 11
# speedup vs baseline: 1.5764x; 1.1052x over previous
"""Trainium2 Bass kernel for the PraxisMemory scatter_memory problem.

Strategy (8 NeuronCores, SPMD single launch):
  - Vocab-sharded fp16 logits GEMM (brain_w.T shard [512,4000] per core) with
    on-device exp + accumulation -> per-core partial sum-exp per token.
    Target-logit dot product kept in exact f32 on the vector engine.
  - Token-similarity matrix (sims) row-sharded across cores via float32r matmuls.
  - Storage MLP applied to every query token row (row-sharded, 64 rows + zero
    rows per core) so the data-dependent retrieval later is a pure gather.
  - Host (numpy, f64) does the tiny decision logic: logsumexp combine, windowed
    threshold, boundary refinement via prefix-sum modularity/conductance deltas,
    event packing into memory slots, top-k retrieval, and final concat.
"""

import os
import numpy as np

import concourse.bacc as bacc
import concourse.mybir as mybir
from concourse.tile import TileContext
from concourse.bass_utils import run_bass_kernel_spmd

F32 = mybir.dt.float32
F32R = mybir.dt.float32r
F16 = mybir.dt.float16
F8 = mybir.dt.float8e4
AF = mybir.ActivationFunctionType
AX = mybir.AxisListType

B, S, D, V = 2, 256, 512, 32000
W_WIN, GAMMA = 20, 2.0
MEM_LEN, NUM_MEM, K_SIM, K_CONT = 16, 256, 8, 4
NCORES = 8
VSH = V // NCORES           # 4000 vocab per core
NV, VT = 8, 512             # vocab tiles per core x tile width (zero-padded)
VPAD = NV * VT - VSH        # 96 zero-padded vocab columns per core
TOK = B * S                 # 512 tokens
RSH = TOK // NCORES         # 64 token rows per core (MLP/tgt shard)
SSH = S // NCORES           # 32 sims rows per batch per core
NR = RSH + 4                # MLP rows per core: 64 query rows + 4 zero rows

_NC = None
LAST_RESULTS = None


def _build_nc():
    nc = bacc.Bacc("TRN2", target_bir_lowering=False, debug=False)

    # inputs (consolidated to minimize DMA instruction count)
    wt8 = nc.dram_tensor("wt8", [NV, 2, 128, 2 * VT], F8, kind="ExternalInput")
    qt8 = nc.dram_tensor("qt8", [2, 128, 2 * TOK], F8, kind="ExternalInput")
    qtf = nc.dram_tensor("qtf", [4, 128, TOK], F32R, kind="ExternalInput")
    qts = nc.dram_tensor("qts", [4, 128, 2 * SSH], F32R, kind="ExternalInput")
    mxt = nc.dram_tensor("mxt", [4, 128, NR], F32R, kind="ExternalInput")
    w1t = nc.dram_tensor("w1t", [4, 128, D], F32R, kind="ExternalInput")
    w2t = nc.dram_tensor("w2t", [4, 128, D], F32R, kind="ExternalInput")
    bb = nc.dram_tensor("bb", [1, 2 * D + 128], F32R, kind="ExternalInput")
    qg = nc.dram_tensor("qg", [RSH, 2 * D], F32, kind="ExternalInput")

    # outputs
    setl_o = nc.dram_tensor("setl_o", [128, 5], F32, kind="ExternalOutput")
    sims_o = nc.dram_tensor("sims_o", [B, SSH, S], F32, kind="ExternalOutput")
    y_o = nc.dram_tensor("y_o", [NR, D], F32, kind="ExternalOutput")

    with TileContext(nc) as tc:
        with tc.tile_pool(name="const", bufs=1) as cpool, \
             tc.tile_pool(name="wts", bufs=2) as wpool, \
             tc.tile_pool(name="work", bufs=3) as work, \
             tc.tile_pool(name="psum", bufs=2, space="PSUM") as psp:

            # ---- logits-critical loads first (HWDGE ring is FIFO) ----
            qt8_sb = cpool.tile([128, 2 * 2 * TOK], F8, tag="qt8")
            nc.sync.dma_start(
                qt8_sb[:].rearrange("p (k t) -> p k t", k=2),
                qt8[:].rearrange("k p t -> p k t"))
            wt_sb = []
            for n in range(NV):
                wt = wpool.tile([128, 2 * 2 * VT], F8, tag=f"wt_{n % 2}",
                                name=f"wt_{n}")
                nc.sync.dma_start(
                    wt[:].rearrange("p (k t) -> p k t", k=2),
                    wt8[n].rearrange("k p t -> p k t"))
                wt_sb.append(wt)

            # small loads after the logits-critical weight stream
            qtf_sb = cpool.tile([128, 4 * TOK], F32R, tag="qtf")
            nc.sync.dma_start(
                qtf_sb[:].rearrange("p (k t) -> p k t", k=4),
                qtf[:].rearrange("k p t -> p k t"))
            qts_sb = cpool.tile([128, 4 * 2 * SSH], F32R, tag="qts")
            nc.sync.dma_start(
                qts_sb[:].rearrange("p (k t) -> p k t", k=4),
                qts[:].rearrange("k p t -> p k t"))
            mxt_sb = cpool.tile([128, 4 * NR], F32R, tag="mxt")
            nc.sync.dma_start(
                mxt_sb[:].rearrange("p (k t) -> p k t", k=4),
                mxt[:].rearrange("k p t -> p k t"))
            w1t_sb = cpool.tile([128, 4 * D], F32R, tag="w1t")
            nc.sync.dma_start(
                w1t_sb[:].rearrange("p (k t) -> p k t", k=4),
                w1t[:].rearrange("k p t -> p k t"))
            w2t_sb = cpool.tile([128, 4 * D], F32R, tag="w2t")
            nc.sync.dma_start(
                w2t_sb[:].rearrange("p (k t) -> p k t", k=4),
                w2t[:].rearrange("k p t -> p k t"))
            bb_sb = cpool.tile([1, 2 * D + 128], F32R, tag="bb")
            nc.sync.dma_start(bb_sb[:], bb[:])
            qg_sb = cpool.tile([RSH, 2 * D], F32, tag="qg")
            nc.sync.dma_start(qg_sb[:], qg[:])

            # ---- logits GEMM (fp8 DoubleRow) + exp-accumulate ----
            separts = [cpool.tile([128, 2], F32, tag=f"sep_{m}",
                                  name=f"sep_{m}") for m in range(4)]
            for g in range(2):
                for m in range(4):
                    ps = psp.tile([128, 4 * VT], F32, tag="lg")
                    for h in range(4):
                        n = g * 4 + h
                        for kk in range(2):
                            lv = qt8_sb[:, kk * 2 * TOK:(kk + 1) * 2 * TOK]
                            lv = lv.rearrange("p (two t) -> p two t", two=2)
                            rv = wt_sb[n][:, kk * 2 * VT:(kk + 1) * 2 * VT]
                            rv = rv.rearrange("p (two v) -> p two v", two=2)
                            nc.tensor.matmul(
                                ps[:, h * VT:(h + 1) * VT],
                                lhsT=lv[:, :, m * 128:(m + 1) * 128],
                                rhs=rv,
                                start=(kk == 0), stop=(kk == 1),
                                perf_mode=mybir.MatmulPerfMode.DoubleRow)
                    et = work.tile([128, 4 * VT], F32, tag="exp")
                    nc.scalar.activation(et[:], ps[:], AF.Exp,
                                         accum_out=separts[m][:, g:g + 1])

            # combined [128,5] output: cols 0-3 sum-exp per token tile,
            # col 4 rows 0-63 target logits
            setl_sb = cpool.tile([128, 5], F32, tag="setl")
            nc.gpsimd.memset(setl_sb[:, 4:5], 0.0)
            for m in range(4):
                nc.vector.reduce_sum(setl_sb[:, m:m + 1], separts[m][:],
                                     axis=AX.X)

            # ---- target-logit dot (exact f32 on DVE) ----
            prod = work.tile([RSH, D], F32, tag="prod")
            nc.vector.tensor_mul(prod[:], qg_sb[:, :D], qg_sb[:, D:])
            nc.vector.reduce_sum(setl_sb[:RSH, 4:5], prod[:], axis=AX.X)
            nc.sync.dma_start(setl_o[:], setl_sb[:])

            # ---- sims row-shard (f32r) ----
            for b in range(B):
                pss = psp.tile([SSH, S], F32, tag="lg", name="pss")
                for k in range(4):
                    nc.tensor.matmul(
                        pss[:],
                        lhsT=qts_sb[:, k * 2 * SSH + b * SSH:
                                    k * 2 * SSH + (b + 1) * SSH],
                        rhs=qtf_sb[:, k * TOK + b * S:k * TOK + (b + 1) * S],
                        start=(k == 0), stop=(k == 3))
                so = work.tile([SSH, S], F32, tag="so")
                nc.vector.tensor_copy(so[:], pss[:])
                nc.sync.dma_start(sims_o[b], so[:])

            # ---- storage MLP on 64 query rows + zero rows ----
            ht_sb = []
            for e in range(4):
                psh = psp.tile([128, NR], F32, tag="lg", name="psh")
                for k in range(4):
                    nc.tensor.matmul(
                        psh[:],
                        lhsT=w1t_sb[:, k * D + e * 128:k * D + (e + 1) * 128],
                        rhs=mxt_sb[:, k * NR:(k + 1) * NR],
                        start=(k == 0), stop=False)
                nc.tensor.matmul(psh[:], lhsT=bb_sb[:, e * 128:(e + 1) * 128],
                                 rhs=bb_sb[:, 2 * D:2 * D + NR],
                                 start=False, stop=True)
                hf = work.tile([128, NR], F32, tag="hf")
                nc.scalar.activation(hf[:], psh[:], AF.Relu)
                ht = cpool.tile([128, NR], F32R, tag=f"ht_{e}", name=f"ht_{e}")
                nc.vector.tensor_copy(ht[:], hf[:])
                ht_sb.append(ht)
            psy = psp.tile([NR, D], F32, tag="lg", name="psy")
            for e in range(4):
                nc.tensor.matmul(psy[:], lhsT=ht_sb[e][:],
                                 rhs=w2t_sb[:, e * D:(e + 1) * D],
                                 start=(e == 0), stop=False)
            nc.tensor.matmul(psy[:], lhsT=bb_sb[:, 2 * D:2 * D + NR],
                             rhs=bb_sb[:, D:2 * D], start=False, stop=True)
            y_sb = work.tile([NR, D], F32, tag="y")
            nc.vector.tensor_copy(y_sb[:], psy[:])
            nc.sync.dma_start(y_o[:], y_sb[:])

    nc.compile()
    return nc


def _get_nc():
    global _NC
    if _NC is None:
        _NC = _build_nc()
    return _NC


# ---------------------------------------------------------------- host logic

def _windowed_threshold(sur):
    wv = np.lib.stride_tricks.sliding_window_view(sur, W_WIN, axis=1)
    t = wv.mean(-1) + GAMMA * wv.std(-1, ddof=1)
    return np.concatenate([np.repeat(t[:, :1], W_WIN - 1, axis=1), t], axis=1)


def _refine(sims, bnd):
    """Candidate boundary-removal decisions via analytic merge deltas (f64)."""
    refined = bnd.copy()
    for b in range(B):
        sim = sims[b].astype(np.float64)
        bd = bnd[b]
        deg = sim.sum(1)
        tot = deg.sum()
        com = np.cumsum(bd).astype(np.int64)
        nlast = int(com[-1])
        mask = (com[:, None] == com[None, :])
        expd = deg[:, None] * deg[None, :] / (tot + 1e-10)
        curQ = ((sim - expd) * mask).sum() / (tot + 1e-10)
        nmax = nlast + 1
        vol = np.zeros(nmax)
        intra = np.zeros(nmax)
        for c in np.unique(com):
            idx = com == c
            vol[c] = deg[idx].sum()
            intra[c] = sim[np.ix_(idx, idx)].sum()
        cuts = vol - intra
        minv = np.minimum(vol, tot - vol)
        cond = np.where(vol > 0, cuts / (minv + 1e-10), 0.0)
        cond_sum = cond.sum()
        curC = cond_sum / (nlast + 1.0)
        for t in np.flatnonzero(bd > 0):
            c = int(com[t])
            A = com == (c - 1)
            Bm = com == c
            S_AB = sim[np.ix_(A, Bm)].sum()
            degA = deg[A].sum()
            degB = deg[Bm].sum()
            newQ = curQ + 2.0 * (S_AB - degA * degB / (tot + 1e-10)) / (tot + 1e-10)
            volM = vol[c - 1] + vol[c]
            intraM = intra[c - 1] + intra[c] + 2.0 * S_AB
            condM = (volM - intraM) / (np.minimum(volM, tot - volM) + 1e-10)
            newC = (cond_sum - cond[c - 1] - cond[c] + condM) / float(nlast)
            if (newQ > curQ) or (newC < curC):
                refined[b, t] = 0.0
    return refined


def _topk_idx(values, k):
    """jax.lax.top_k semantics: descending values, ties -> lowest index."""
    n = len(values)
    order = np.lexsort((np.arange(n), -values))
    return order[:k]


def kernel(**inputs):
    global LAST_RESULTS
    query = np.asarray(inputs["query"], dtype=np.float32)
    key = np.asarray(inputs["key"], dtype=np.float32)
    attention_mask = np.asarray(inputs["attention_mask"], dtype=np.float32)
    target_tokens = np.asarray(inputs["target_tokens"]).astype(np.int64)
    brain_w = np.asarray(inputs["brain_w"], dtype=np.float32)
    storage_w1 = np.asarray(inputs["storage_w1"], dtype=np.float32)
    storage_b1 = np.asarray(inputs["storage_b1"], dtype=np.float32)
    storage_w2 = np.asarray(inputs["storage_w2"], dtype=np.float32)
    storage_b2 = np.asarray(inputs["storage_b2"], dtype=np.float32)
    sim_w = np.asarray(inputs["sim_w"], dtype=np.float32)
    sim_b = np.asarray(inputs["sim_b"], dtype=np.float32)
    memory_keys = np.asarray(inputs["memory_keys"], dtype=np.float32)
    memory_values = np.asarray(inputs["memory_values"], dtype=np.float32)
    memory_lengths = np.asarray(inputs["memory_lengths"])

    qflat = np.ascontiguousarray(query.reshape(TOK, D))
    qT = np.ascontiguousarray(qflat.T)                      # [D, TOK] f32
    tflat = target_tokens.reshape(-1)

    import ml_dtypes
    F8NP = ml_dtypes.float8_e4m3
    qt8_np = np.ascontiguousarray(
        qT.astype(F8NP).reshape(2, 2, 128, TOK).transpose(0, 2, 1, 3)
    ).reshape(2, 128, 2 * TOK)
    qtf_np = qT.reshape(4, 128, TOK)
    w1t_np = np.ascontiguousarray(storage_w1.T).reshape(4, 128, D)
    w2t_np = np.ascontiguousarray(storage_w2.T).reshape(4, 128, D)
    bb_np = np.concatenate(
        [storage_b1, storage_b2, np.ones(128, np.float32)])[None, :]
    w8tp = np.zeros((NCORES, D, NV * VT), F8NP)             # padded W.T shards
    wt8_all = brain_w.astype(F8NP)

    in_maps = []
    for c in range(NCORES):
        c0 = c * VSH
        w8tp[c][:, :VSH] = wt8_all[c0:c0 + VSH].T
        wt = np.ascontiguousarray(
            w8tp[c].reshape(2, 2, 128, NV, VT).transpose(3, 0, 2, 1, 4)
        ).reshape(NV, 2, 128, 2 * VT)
        r0 = c * SSH
        qts_np = np.ascontiguousarray(np.concatenate(
            [qT[:, r0:r0 + SSH], qT[:, S + r0:S + r0 + SSH]], axis=1)
        ).reshape(4, 128, 2 * SSH)
        t0 = c * RSH
        mxt_np = np.ascontiguousarray(np.concatenate(
            [qT[:, t0:t0 + RSH], np.zeros((D, NR - RSH), np.float32)], axis=1)
        ).reshape(4, 128, NR)
        qg_np = np.concatenate(
            [qflat[t0:t0 + RSH], brain_w[tflat[t0:t0 + RSH]]], axis=1)
        in_maps.append(dict(
            wt8=wt, qt8=qt8_np, qtf=qtf_np, qts=qts_np, mxt=mxt_np,
            w1t=w1t_np, w2t=w2t_np, bb=bb_np,
            qg=np.ascontiguousarray(qg_np),
        ))

    trace = bool(os.environ.get("KBENCH_TRACE"))
    res = run_bass_kernel_spmd(_get_nc(), in_maps, list(range(NCORES)),
                               trace=trace)
    LAST_RESULTS = res
    dev = res.results

    # ---- combine: surprise ----
    se = np.zeros(TOK, np.float64)
    tl = np.zeros(TOK, np.float64)
    for c in range(NCORES):
        se += dev[c]["setl_o"][:, :4].astype(np.float64).T.reshape(-1)
        tl[c * RSH:(c + 1) * RSH] = dev[c]["setl_o"][:RSH, 4]
    se -= float(NCORES * VPAD)        # exp(0)=1 per zero-padded vocab column
    surprise = (np.log(se) - tl).reshape(B, S)

    thr = _windowed_threshold(surprise)
    boundaries = (surprise > thr).astype(np.float64)

    sims = np.zeros((B, S, S), np.float32)
    for c in range(NCORES):
        sims[:, c * SSH:(c + 1) * SSH, :] = dev[c]["sims_o"]

    refined = _refine(sims, boundaries)

    # ---- event packing into memory slots ----
    labels = np.cumsum(refined, axis=1).astype(np.int64)
    events = []                                    # rank order: (b, s0, cnt)
    for b in range(B):
        lab = labels[b]
        starts = np.concatenate([[0], np.flatnonzero(np.diff(lab)) + 1])
        counts = np.diff(np.concatenate([starts, [S]]))
        for s0, cnt in zip(starts, counts):
            events.append((b, int(s0), int(cnt)))
    written = {}                                   # slot -> (b, s0, cnt)
    for rank, ev in enumerate(events):
        written[rank % NUM_MEM] = ev               # last write wins on wrap

    m_lens = memory_lengths.astype(np.int64).copy()
    valid = m_lens > 0
    key_vec = memory_keys.astype(np.float64).copy()
    for slot, (b, s0, cnt) in written.items():
        valid[slot] = True
        key_vec[slot] = qflat[b * S + s0]

    # ---- retrieval ----
    q64 = qflat.astype(np.float64).reshape(B, S, D)
    qm = q64.mean(1) @ sim_w.astype(np.float64).T + sim_b.astype(np.float64)
    scores = qm @ key_vec.T                        # [B, NUM_MEM]
    scores = np.where(valid[None, :], scores, -1e9)
    top_i = np.stack([_topk_idx(scores[b], K_SIM) for b in range(B)])
    rec = np.where(valid, np.arange(NUM_MEM), -1)
    cont_i = _topk_idx(rec.astype(np.float64), K_CONT)

    # ---- gather storage() rows ----
    Y = np.zeros((TOK, D), np.float32)
    for c in range(NCORES):
        Y[c * RSH:(c + 1) * RSH] = dev[c]["y_o"][:RSH]
    z_row = dev[0]["y_o"][RSH]

    def host_storage(x):
        h = np.maximum(x @ storage_w1.T + storage_b1, 0.0)
        return h @ storage_w2.T + storage_b2

    def slot_rows(slot):
        out = np.empty((MEM_LEN, D), np.float32)
        if slot in written:
            b, s0, cnt = written[slot]
            n = min(cnt, MEM_LEN)
            out[:n] = Y[b * S + s0:b * S + s0 + n]
            out[n:] = z_row
        else:
            mv = memory_values[slot]
            nz = np.any(mv != 0, axis=1)
            out[:] = z_row
            if nz.any():
                out[nz] = host_storage(mv[nz]).astype(np.float32)
        return out

    sim_part = np.stack([
        np.concatenate([slot_rows(s) for s in top_i[b]], axis=0)
        for b in range(B)])                        # [B, 128, D]
    cont_part = np.concatenate([slot_rows(s) for s in cont_i], axis=0)  # [64,D]

    context = np.concatenate(
        [sim_part, np.broadcast_to(cont_part[None], (B, K_CONT * MEM_LEN, D)),
         key], axis=1).astype(np.float32)
    ext_mask = np.concatenate(
        [np.ones((B, (K_SIM + K_CONT) * MEM_LEN), attention_mask.dtype),
         attention_mask], axis=1)

    return query, context, context, ext_mask


# revision 12
# speedup vs baseline: 1.8602x; 1.1800x over previous
"""Trainium2 Bass kernel for the PraxisMemory scatter_memory problem.

Strategy (8 NeuronCores, SPMD single launch):
  - Vocab-sharded fp16 logits GEMM (brain_w.T shard [512,4000] per core) with
    on-device exp + accumulation -> per-core partial sum-exp per token.
    Target-logit dot product kept in exact f32 on the vector engine.
  - Token-similarity matrix (sims) row-sharded across cores via float32r matmuls.
  - Storage MLP applied to every query token row (row-sharded, 64 rows + zero
    rows per core) so the data-dependent retrieval later is a pure gather.
  - Host (numpy, f64) does the tiny decision logic: logsumexp combine, windowed
    threshold, boundary refinement via prefix-sum modularity/conductance deltas,
    event packing into memory slots, top-k retrieval, and final concat.
"""

import os
import numpy as np

import concourse.bacc as bacc
import concourse.mybir as mybir
from concourse.tile import TileContext
from concourse.bass_utils import run_bass_kernel_spmd

F32 = mybir.dt.float32
F32R = mybir.dt.float32r
F16 = mybir.dt.float16
F8 = mybir.dt.float8e4
AF = mybir.ActivationFunctionType
AX = mybir.AxisListType

B, S, D, V = 2, 256, 512, 32000
W_WIN, GAMMA = 20, 2.0
MEM_LEN, NUM_MEM, K_SIM, K_CONT = 16, 256, 8, 4
NCORES = 8
VSH = V // NCORES           # 4000 vocab per core
NV, VT = 8, 512             # vocab tiles per core x tile width (zero-padded)
VPAD = NV * VT - VSH        # 96 zero-padded vocab columns per core
TOK = B * S                 # 512 tokens
RSH = TOK // NCORES         # 64 token rows per core (MLP/tgt shard)
SSH = S // NCORES           # 32 sims rows per batch per core
NR = RSH + 4                # MLP rows per core: 64 query rows + 4 zero rows

_NC = None
LAST_RESULTS = None


def _build_nc():
    nc = bacc.Bacc("TRN2", target_bir_lowering=False, debug=False)

    # inputs (consolidated to minimize DMA instruction count)
    wt8 = nc.dram_tensor("wt8", [NV, 2, 128, 2 * VT], F8, kind="ExternalInput")
    qt8 = nc.dram_tensor("qt8", [2, 128, 2 * TOK], F8, kind="ExternalInput")
    qtf = nc.dram_tensor("qtf", [4, 128, TOK], F32R, kind="ExternalInput")
    qts = nc.dram_tensor("qts", [4, 128, 2 * SSH], F32R, kind="ExternalInput")
    mxt = nc.dram_tensor("mxt", [4, 128, NR], F32R, kind="ExternalInput")
    w1t = nc.dram_tensor("w1t", [4, 128, D], F32R, kind="ExternalInput")
    w2t = nc.dram_tensor("w2t", [4, 128, D], F32R, kind="ExternalInput")
    bb = nc.dram_tensor("bb", [1, 2 * D + 128], F32R, kind="ExternalInput")
    qg = nc.dram_tensor("qg", [RSH, 2 * D], F32, kind="ExternalInput")

    # outputs
    setl_o = nc.dram_tensor("setl_o", [128, 5], F32, kind="ExternalOutput")
    sims_o = nc.dram_tensor("sims_o", [B, SSH, S], F32, kind="ExternalOutput")
    y_o = nc.dram_tensor("y_o", [NR, D], F32, kind="ExternalOutput")

    with TileContext(nc) as tc:
        with tc.tile_pool(name="const", bufs=1) as cpool, \
             tc.tile_pool(name="wts", bufs=1) as wpool, \
             tc.tile_pool(name="work", bufs=3) as work, \
             tc.tile_pool(name="psum", bufs=2, space="PSUM") as psp:

            # ---- logits-critical loads first (HWDGE ring is FIFO) ----
            qt8_sb = cpool.tile([128, 2 * 2 * TOK], F8, tag="qt8")
            nc.sync.dma_start(
                qt8_sb[:].rearrange("p (k t) -> p k t", k=2),
                qt8[:].rearrange("k p t -> p k t"))
            wt_sb = []
            for n in range(NV):
                wt = wpool.tile([128, 2 * 2 * VT], F8, tag=f"wt_{n}",
                                name=f"wt_{n}")
                nc.sync.dma_start(
                    wt[:].rearrange("p (k t) -> p k t", k=2),
                    wt8[n].rearrange("k p t -> p k t"))
                wt_sb.append(wt)

            # small loads after the logits-critical weight stream
            qtf_sb = cpool.tile([128, 4 * TOK], F32R, tag="qtf")
            nc.sync.dma_start(
                qtf_sb[:].rearrange("p (k t) -> p k t", k=4),
                qtf[:].rearrange("k p t -> p k t"))
            qts_sb = cpool.tile([128, 4 * 2 * SSH], F32R, tag="qts")
            nc.sync.dma_start(
                qts_sb[:].rearrange("p (k t) -> p k t", k=4),
                qts[:].rearrange("k p t -> p k t"))
            mxt_sb = cpool.tile([128, 4 * NR], F32R, tag="mxt")
            nc.sync.dma_start(
                mxt_sb[:].rearrange("p (k t) -> p k t", k=4),
                mxt[:].rearrange("k p t -> p k t"))
            w1t_sb = cpool.tile([128, 4 * D], F32R, tag="w1t")
            nc.sync.dma_start(
                w1t_sb[:].rearrange("p (k t) -> p k t", k=4),
                w1t[:].rearrange("k p t -> p k t"))
            w2t_sb = cpool.tile([128, 4 * D], F32R, tag="w2t")
            nc.sync.dma_start(
                w2t_sb[:].rearrange("p (k t) -> p k t", k=4),
                w2t[:].rearrange("k p t -> p k t"))
            bb_sb = cpool.tile([1, 2 * D + 128], F32R, tag="bb")
            nc.sync.dma_start(bb_sb[:], bb[:])
            qg_sb = cpool.tile([RSH, 2 * D], F32, tag="qg")
            nc.sync.dma_start(qg_sb[:], qg[:])

            # ---- logits GEMM (fp8 DoubleRow) + exp-accumulate ----
            separts = [cpool.tile([128, 2], F32, tag=f"sep_{m}",
                                  name=f"sep_{m}") for m in range(4)]
            for g in range(2):
                for m in range(4):
                    ps = psp.tile([128, 4 * VT], F32, tag="lg")
                    for h in range(4):
                        n = g * 4 + h
                        for kk in range(2):
                            lv = qt8_sb[:, kk * 2 * TOK:(kk + 1) * 2 * TOK]
                            lv = lv.rearrange("p (two t) -> p two t", two=2)
                            rv = wt_sb[n][:, kk * 2 * VT:(kk + 1) * 2 * VT]
                            rv = rv.rearrange("p (two v) -> p two v", two=2)
                            nc.tensor.matmul(
                                ps[:, h * VT:(h + 1) * VT],
                                lhsT=lv[:, :, m * 128:(m + 1) * 128],
                                rhs=rv,
                                start=(kk == 0), stop=(kk == 1),
                                perf_mode=mybir.MatmulPerfMode.DoubleRow)
                    et = work.tile([128, 4 * VT], F32, tag="exp")
                    nc.scalar.activation(et[:], ps[:], AF.Exp,
                                         accum_out=separts[m][:, g:g + 1])

            # combined [128,5] output: cols 0-3 sum-exp per token tile,
            # col 4 rows 0-63 target logits
            setl_sb = cpool.tile([128, 5], F32, tag="setl")
            nc.gpsimd.memset(setl_sb[:, 4:5], 0.0)
            for m in range(4):
                nc.vector.reduce_sum(setl_sb[:, m:m + 1], separts[m][:],
                                     axis=AX.X)

            # ---- target-logit dot (exact f32 on DVE) ----
            prod = work.tile([RSH, D], F32, tag="prod")
            nc.vector.tensor_mul(prod[:], qg_sb[:, :D], qg_sb[:, D:])
            nc.vector.reduce_sum(setl_sb[:RSH, 4:5], prod[:], axis=AX.X)
            nc.sync.dma_start(setl_o[:], setl_sb[:])

            # ---- sims row-shard (f32r) ----
            for b in range(B):
                pss = psp.tile([SSH, S], F32, tag="lg", name="pss")
                for k in range(4):
                    nc.tensor.matmul(
                        pss[:],
                        lhsT=qts_sb[:, k * 2 * SSH + b * SSH:
                                    k * 2 * SSH + (b + 1) * SSH],
                        rhs=qtf_sb[:, k * TOK + b * S:k * TOK + (b + 1) * S],
                        start=(k == 0), stop=(k == 3))
                so = work.tile([SSH, S], F32, tag="so")
                nc.vector.tensor_copy(so[:], pss[:])
                nc.sync.dma_start(sims_o[b], so[:])

            # ---- storage MLP on 64 query rows + zero rows ----
            ht_sb = []
            for e in range(4):
                psh = psp.tile([128, NR], F32, tag="lg", name="psh")
                for k in range(4):
                    nc.tensor.matmul(
                        psh[:],
                        lhsT=w1t_sb[:, k * D + e * 128:k * D + (e + 1) * 128],
                        rhs=mxt_sb[:, k * NR:(k + 1) * NR],
                        start=(k == 0), stop=False)
                nc.tensor.matmul(psh[:], lhsT=bb_sb[:, e * 128:(e + 1) * 128],
                                 rhs=bb_sb[:, 2 * D:2 * D + NR],
                                 start=False, stop=True)
                hf = work.tile([128, NR], F32, tag="hf")
                nc.scalar.activation(hf[:], psh[:], AF.Relu)
                ht = cpool.tile([128, NR], F32R, tag=f"ht_{e}", name=f"ht_{e}")
                nc.vector.tensor_copy(ht[:], hf[:])
                ht_sb.append(ht)
            psy = psp.tile([NR, D], F32, tag="lg", name="psy")
            for e in range(4):
                nc.tensor.matmul(psy[:], lhsT=ht_sb[e][:],
                                 rhs=w2t_sb[:, e * D:(e + 1) * D],
                                 start=(e == 0), stop=False)
            nc.tensor.matmul(psy[:], lhsT=bb_sb[:, 2 * D:2 * D + NR],
                             rhs=bb_sb[:, D:2 * D], start=False, stop=True)
            y_sb = work.tile([NR, D], F32, tag="y")
            nc.vector.tensor_copy(y_sb[:], psy[:])
            nc.sync.dma_start(y_o[:], y_sb[:])

    nc.compile()
    return nc


def _get_nc():
    global _NC
    if _NC is None:
        _NC = _build_nc()
    return _NC


# ---------------------------------------------------------------- host logic

def _windowed_threshold(sur):
    wv = np.lib.stride_tricks.sliding_window_view(sur, W_WIN, axis=1)
    t = wv.mean(-1) + GAMMA * wv.std(-1, ddof=1)
    return np.concatenate([np.repeat(t[:, :1], W_WIN - 1, axis=1), t], axis=1)


def _refine(sims, bnd):
    """Candidate boundary-removal decisions via analytic merge deltas (f64)."""
    refined = bnd.copy()
    for b in range(B):
        sim = sims[b].astype(np.float64)
        bd = bnd[b]
        deg = sim.sum(1)
        tot = deg.sum()
        com = np.cumsum(bd).astype(np.int64)
        nlast = int(com[-1])
        mask = (com[:, None] == com[None, :])
        expd = deg[:, None] * deg[None, :] / (tot + 1e-10)
        curQ = ((sim - expd) * mask).sum() / (tot + 1e-10)
        nmax = nlast + 1
        vol = np.zeros(nmax)
        intra = np.zeros(nmax)
        for c in np.unique(com):
            idx = com == c
            vol[c] = deg[idx].sum()
            intra[c] = sim[np.ix_(idx, idx)].sum()
        cuts = vol - intra
        minv = np.minimum(vol, tot - vol)
        cond = np.where(vol > 0, cuts / (minv + 1e-10), 0.0)
        cond_sum = cond.sum()
        curC = cond_sum / (nlast + 1.0)
        for t in np.flatnonzero(bd > 0):
            c = int(com[t])
            A = com == (c - 1)
            Bm = com == c
            S_AB = sim[np.ix_(A, Bm)].sum()
            degA = deg[A].sum()
            degB = deg[Bm].sum()
            newQ = curQ + 2.0 * (S_AB - degA * degB / (tot + 1e-10)) / (tot + 1e-10)
            volM = vol[c - 1] + vol[c]
            intraM = intra[c - 1] + intra[c] + 2.0 * S_AB
            condM = (volM - intraM) / (np.minimum(volM, tot - volM) + 1e-10)
            newC = (cond_sum - cond[c - 1] - cond[c] + condM) / float(nlast)
            if (newQ > curQ) or (newC < curC):
                refined[b, t] = 0.0
    return refined


def _topk_idx(values, k):
    """jax.lax.top_k semantics: descending values, ties -> lowest index."""
    n = len(values)
    order = np.lexsort((np.arange(n), -values))
    return order[:k]


def kernel(**inputs):
    global LAST_RESULTS
    query = np.asarray(inputs["query"], dtype=np.float32)
    key = np.asarray(inputs["key"], dtype=np.float32)
    attention_mask = np.asarray(inputs["attention_mask"], dtype=np.float32)
    target_tokens = np.asarray(inputs["target_tokens"]).astype(np.int64)
    brain_w = np.asarray(inputs["brain_w"], dtype=np.float32)
    storage_w1 = np.asarray(inputs["storage_w1"], dtype=np.float32)
    storage_b1 = np.asarray(inputs["storage_b1"], dtype=np.float32)
    storage_w2 = np.asarray(inputs["storage_w2"], dtype=np.float32)
    storage_b2 = np.asarray(inputs["storage_b2"], dtype=np.float32)
    sim_w = np.asarray(inputs["sim_w"], dtype=np.float32)
    sim_b = np.asarray(inputs["sim_b"], dtype=np.float32)
    memory_keys = np.asarray(inputs["memory_keys"], dtype=np.float32)
    memory_values = np.asarray(inputs["memory_values"], dtype=np.float32)
    memory_lengths = np.asarray(inputs["memory_lengths"])

    qflat = np.ascontiguousarray(query.reshape(TOK, D))
    qT = np.ascontiguousarray(qflat.T)                      # [D, TOK] f32
    tflat = target_tokens.reshape(-1)

    import ml_dtypes
    F8NP = ml_dtypes.float8_e4m3
    qt8_np = np.ascontiguousarray(
        qT.astype(F8NP).reshape(2, 2, 128, TOK).transpose(0, 2, 1, 3)
    ).reshape(2, 128, 2 * TOK)
    qtf_np = qT.reshape(4, 128, TOK)
    w1t_np = np.ascontiguousarray(storage_w1.T).reshape(4, 128, D)
    w2t_np = np.ascontiguousarray(storage_w2.T).reshape(4, 128, D)
    bb_np = np.concatenate(
        [storage_b1, storage_b2, np.ones(128, np.float32)])[None, :]
    w8tp = np.zeros((NCORES, D, NV * VT), F8NP)             # padded W.T shards
    wt8_all = brain_w.astype(F8NP)

    in_maps = []
    for c in range(NCORES):
        c0 = c * VSH
        w8tp[c][:, :VSH] = wt8_all[c0:c0 + VSH].T
        wt = np.ascontiguousarray(
            w8tp[c].reshape(2, 2, 128, NV, VT).transpose(3, 0, 2, 1, 4)
        ).reshape(NV, 2, 128, 2 * VT)
        r0 = c * SSH
        qts_np = np.ascontiguousarray(np.concatenate(
            [qT[:, r0:r0 + SSH], qT[:, S + r0:S + r0 + SSH]], axis=1)
        ).reshape(4, 128, 2 * SSH)
        t0 = c * RSH
        mxt_np = np.ascontiguousarray(np.concatenate(
            [qT[:, t0:t0 + RSH], np.zeros((D, NR - RSH), np.float32)], axis=1)
        ).reshape(4, 128, NR)
        qg_np = np.concatenate(
            [qflat[t0:t0 + RSH], brain_w[tflat[t0:t0 + RSH]]], axis=1)
        in_maps.append(dict(
            wt8=wt, qt8=qt8_np, qtf=qtf_np, qts=qts_np, mxt=mxt_np,
            w1t=w1t_np, w2t=w2t_np, bb=bb_np,
            qg=np.ascontiguousarray(qg_np),
        ))

    trace = bool(os.environ.get("KBENCH_TRACE"))
    res = run_bass_kernel_spmd(_get_nc(), in_maps, list(range(NCORES)),
                               trace=trace)
    LAST_RESULTS = res
    dev = res.results

    # ---- combine: surprise ----
    se = np.zeros(TOK, np.float64)
    tl = np.zeros(TOK, np.float64)
    for c in range(NCORES):
        se += dev[c]["setl_o"][:, :4].astype(np.float64).T.reshape(-1)
        tl[c * RSH:(c + 1) * RSH] = dev[c]["setl_o"][:RSH, 4]
    se -= float(NCORES * VPAD)        # exp(0)=1 per zero-padded vocab column
    surprise = (np.log(se) - tl).reshape(B, S)

    thr = _windowed_threshold(surprise)
    boundaries = (surprise > thr).astype(np.float64)

    sims = np.zeros((B, S, S), np.float32)
    for c in range(NCORES):
        sims[:, c * SSH:(c + 1) * SSH, :] = dev[c]["sims_o"]

    refined = _refine(sims, boundaries)

    # ---- event packing into memory slots ----
    labels = np.cumsum(refined, axis=1).astype(np.int64)
    events = []                                    # rank order: (b, s0, cnt)
    for b in range(B):
        lab = labels[b]
        starts = np.concatenate([[0], np.flatnonzero(np.diff(lab)) + 1])
        counts = np.diff(np.concatenate([starts, [S]]))
        for s0, cnt in zip(starts, counts):
            events.append((b, int(s0), int(cnt)))
    written = {}                                   # slot -> (b, s0, cnt)
    for rank, ev in enumerate(events):
        written[rank % NUM_MEM] = ev               # last write wins on wrap

    m_lens = memory_lengths.astype(np.int64).copy()
    valid = m_lens > 0
    key_vec = memory_keys.astype(np.float64).copy()
    for slot, (b, s0, cnt) in written.items():
        valid[slot] = True
        key_vec[slot] = qflat[b * S + s0]

    # ---- retrieval ----
    q64 = qflat.astype(np.float64).reshape(B, S, D)
    qm = q64.mean(1) @ sim_w.astype(np.float64).T + sim_b.astype(np.float64)
    scores = qm @ key_vec.T                        # [B, NUM_MEM]
    scores = np.where(valid[None, :], scores, -1e9)
    top_i = np.stack([_topk_idx(scores[b], K_SIM) for b in range(B)])
    rec = np.where(valid, np.arange(NUM_MEM), -1)
    cont_i = _topk_idx(rec.astype(np.float64), K_CONT)

    # ---- gather storage() rows ----
    Y = np.zeros((TOK, D), np.float32)
    for c in range(NCORES):
        Y[c * RSH:(c + 1) * RSH] = dev[c]["y_o"][:RSH]
    z_row = dev[0]["y_o"][RSH]

    def host_storage(x):
        h = np.maximum(x @ storage_w1.T + storage_b1, 0.0)
        return h @ storage_w2.T + storage_b2

    def slot_rows(slot):
        out = np.empty((MEM_LEN, D), np.float32)
        if slot in written:
            b, s0, cnt = written[slot]
            n = min(cnt, MEM_LEN)
            out[:n] = Y[b * S + s0:b * S + s0 + n]
            out[n:] = z_row
        else:
            mv = memory_values[slot]
            nz = np.any(mv != 0, axis=1)
            out[:] = z_row
            if nz.any():
                out[nz] = host_storage(mv[nz]).astype(np.float32)
        return out

    sim_part = np.stack([
        np.concatenate([slot_rows(s) for s in top_i[b]], axis=0)
        for b in range(B)])                        # [B, 128, D]
    cont_part = np.concatenate([slot_rows(s) for s in cont_i], axis=0)  # [64,D]

    context = np.concatenate(
        [sim_part, np.broadcast_to(cont_part[None], (B, K_CONT * MEM_LEN, D)),
         key], axis=1).astype(np.float32)
    ext_mask = np.concatenate(
        [np.ones((B, (K_SIM + K_CONT) * MEM_LEN), attention_mask.dtype),
         attention_mask], axis=1)

    return query, context, context, ext_mask
